# revision 1
# baseline (speedup 1.0000x reference)
"""Trainium2 Bass kernel for nn_EntInit (gnn_message_passing).

feat[n, :] = mean over incoming edges e (dst[e] == n) of T[etypes[e], :]
where T = concat(rel_head_emb, rel_tail_emb)  [400, 128].

Histogram formulation (no gather, no scatter, no gpsimd):
  Shard NODES across 8 cores (49 blocks of 128 nodes each).  Per core,
  edges are grouped by (node-block, type-chunk) on host; on device, for
  each 128-edge tile build two one-hots (DVE tensor_scalar(is_equal)
  against iota constants; a fraction on the Scalar engine via an exact
  Square+Relu(1-x) identity to balance load):
    OH[e, t] over the 100-wide type chunk, S[e, n] over the 128 block
  nodes.  One PE matmul per tile accumulates A^T[t, n] += OH^T @ S in
  PSUM (exact integer counts).  Per block, 8 small matmuls compute
  sums[n, :] = A @ T_hi + A @ T_lo and count[n] = A @ 1 (bf16 hi/lo
  split of the f32 table keeps products exact), then
  feat = sums * reciprocal(max(count, 1)) is written densely.
  Host work: one stable argsort + index packing; unshard = concat.
"""
import sys

sys.path.insert(0, "/opt/trn_rl_repo")

import numpy as np
import ml_dtypes

import concourse.bass as bass
import concourse.bacc as bacc
import concourse.mybir as mybir
import concourse.tile as tile
from concourse import bass_utils

NUM_REL = 200
N_TYPES = 2 * NUM_REL          # 400 relation rows
N_CORES = 8
P = 128
WC = 100                       # type-chunk width (4 chunks x 100 = 400)
NCH = 4
PAD_ET = 1024.0                # pad etype value: matches no iota column
ACT_MOD = 5                    # every 5th S one-hot built on ScalarE
OH_ACT_PHASE = -1              # 1-in-10 OH one-hots on ScalarE (-1: off)
BF16 = ml_dtypes.bfloat16

_prog_cache: dict = {}
_runner_cache: dict = {}


def _build_program(B: int, Tc: int, repeats: int = 1):
    """One SPMD program; cores differ only in input data.
    B node-blocks of 128 nodes; Tc 128-edge tiles per (block, chunk)."""
    TT = B * NCH * Tc
    nc = bacc.Bacc("TRN2", debug=False, num_devices=1)
    # eds: [etc | dsc] f32; iox: [iota_types | iota_nodes] fp16;
    # tbl: [T_hi|ones , T_lo] bf16 chunk-major.
    eds = nc.dram_tensor("eds", [P, 2 * TT], mybir.dt.float32,
                         kind="ExternalInput").ap()
    iox = nc.dram_tensor("iox", [P, N_TYPES + P], mybir.dt.float16,
                         kind="ExternalInput").ap()
    tbl = nc.dram_tensor("tbl", [WC, NCH * 257], mybir.dt.bfloat16,
                         kind="ExternalInput").ap()
    feat = nc.dram_tensor("feat", [B * P, P], mybir.dt.float32,
                          kind="ExternalOutput").ap()

    with tile.TileContext(nc) as tc:
        with (
            tc.tile_pool(name="const", bufs=1) as const_tp,
            tc.tile_pool(name="oh", bufs=24) as oh_tp,
            tc.tile_pool(name="sel", bufs=24) as s_tp,
            tc.tile_pool(name="at", bufs=8) as at_tp,
            tc.tile_pool(name="norm", bufs=3) as nm_tp,
            tc.tile_pool(name="psA", bufs=3, space="PSUM") as psA_tp,
            tc.tile_pool(name="psS", bufs=3, space="PSUM") as psS_tp,
        ):
            eds_sb = const_tp.tile([P, 2 * TT], mybir.dt.float32)
            iox_sb = const_tp.tile([P, N_TYPES + P], mybir.dt.float16)
            tbl_sb = const_tp.tile([WC, NCH, 257], mybir.dt.bfloat16)
            nc.sync.dma_start(out=eds_sb[:], in_=eds[:])
            nc.sync.dma_start(out=iox_sb[:], in_=iox[:])
            nc.sync.dma_start(out=tbl_sb[:], in_=tbl[:])
            etc_sb = eds_sb[:, 0:TT]
            dsc_sb = eds_sb[:, TT:2 * TT]
            iot_sb = iox_sb[:, 0:N_TYPES]
            ion_sb = iox_sb[:, N_TYPES:N_TYPES + P]

            gi = 0
            for _rep in range(repeats):
              for b in range(B):
                  psA = psA_tp.tile([P, 512], mybir.dt.float32, tag="A")
                  for c in range(NCH):
                      for t in range(Tc):
                          col = (b * NCH + c) * Tc + t
                          oh = oh_tp.tile([P, WC], mybir.dt.bfloat16, tag="oh")
                          if gi % 10 == OH_ACT_PHASE:
                              oq = oh_tp.tile([P, WC], mybir.dt.float16,
                                              tag="oq")
                              nc.scalar.activation(
                                  out=oq[:],
                                  in_=iot_sb[:, c * WC:(c + 1) * WC],
                                  func=mybir.ActivationFunctionType.Square,
                                  bias=etc_sb[:, col:col + 1], scale=-1.0)
                              nc.scalar.activation(
                                  out=oh[:], in_=oq[:],
                                  func=mybir.ActivationFunctionType.Relu,
                                  bias=1.0, scale=-1.0)
                          else:
                              nc.vector.tensor_scalar(
                                  out=oh[:], in0=iot_sb[:, c * WC:(c + 1) * WC],
                                  scalar1=etc_sb[:, col:col + 1], scalar2=None,
                                  op0=mybir.AluOpType.is_equal)
                          s = s_tp.tile([P, P], mybir.dt.bfloat16, tag="s")
                          if gi % ACT_MOD == 0:
                              # ScalarE: s = Relu(1 - (iota - d)^2), exact 0/1
                              sq = s_tp.tile([P, P], mybir.dt.float16, tag="sq")
                              nc.scalar.activation(
                                  out=sq[:], in_=ion_sb[:],
                                  func=mybir.ActivationFunctionType.Square,
                                  bias=dsc_sb[:, col:col + 1], scale=-1.0)
                              nc.scalar.activation(
                                  out=s[:], in_=sq[:],
                                  func=mybir.ActivationFunctionType.Relu,
                                  bias=1.0, scale=-1.0)
                          else:
                              nc.vector.tensor_scalar(
                                  out=s[:], in0=ion_sb[:],
                                  scalar1=dsc_sb[:, col:col + 1], scalar2=None,
                                  op0=mybir.AluOpType.is_equal)
                          gi += 1
                          nc.tensor.matmul(
                              out=psA[0:WC, c * P:(c + 1) * P],
                              lhsT=oh[:], rhs=s[:],
                              start=(t == 0), stop=(t == Tc - 1))
                  psS = psS_tp.tile([P, 129], mybir.dt.float32, tag="S")
                  for c in range(NCH):
                      at = at_tp.tile([P, P], mybir.dt.bfloat16, tag="at")
                      nc.scalar.copy(out=at[0:WC, :],
                                     in_=psA[0:WC, c * P:(c + 1) * P])
                      nc.tensor.matmul(
                          out=psS[:, 0:129], lhsT=at[0:WC, :],
                          rhs=tbl_sb[:, c, 0:129],
                          start=(c == 0), stop=False)
                      nc.tensor.matmul(
                          out=psS[:, 0:128], lhsT=at[0:WC, :],
                          rhs=tbl_sb[:, c, 129:257],
                          start=False, stop=(c == NCH - 1))
                  cm = nm_tp.tile([P, 1], mybir.dt.float32, tag="cm")
                  nc.vector.tensor_scalar(
                      out=cm[:], in0=psS[:, 128:129], scalar1=1.0,
                      scalar2=None, op0=mybir.AluOpType.max)
                  rc = nm_tp.tile([P, 1], mybir.dt.float32, tag="rc")
                  nc.vector.reciprocal(out=rc[:], in_=cm[:])
                  ft = nm_tp.tile([P, P], mybir.dt.float32, tag="ft")
                  nc.scalar.mul(out=ft[:], in_=psS[:, 0:128], mul=rc[:])
                  nc.sync.dma_start(out=feat[b * P:(b + 1) * P, :], in_=ft[:])

    nc.compile()
    return nc


def _host_prepare(et: np.ndarray, d: np.ndarray, n_nodes: int):
    """Group edges by (node-block, type-chunk); pad groups to Tc tiles.
    Returns per-core packed [etc | dsc] arrays."""
    E = et.shape[0]
    nblk = -(-n_nodes // P)
    B = -(-nblk // N_CORES)
    nb_tot = B * N_CORES
    g = (d >> 7) * NCH + et // WC
    NG = nb_tot * NCH
    sizes = np.bincount(g, minlength=NG)
    Tc = max(1, int(-(-int(sizes.max()) // P)))
    cap = Tc * P
    order = np.argsort(g, kind="stable")
    gs = g[order]
    starts = np.zeros(NG + 1, np.int64)
    np.cumsum(sizes, out=starts[1:])
    slot = gs * cap + (np.arange(E, dtype=np.int64) - starts[gs])
    etf = np.full(NG * cap, PAD_ET, np.float32)
    dsf = np.zeros(NG * cap, np.float32)
    etf[slot] = et[order].astype(np.float32)
    dsf[slot] = (d[order] & 127).astype(np.float32)
    TTc = B * NCH * Tc
    arr_e = etf.reshape(N_CORES, TTc, P).transpose(0, 2, 1)
    arr_d = dsf.reshape(N_CORES, TTc, P).transpose(0, 2, 1)
    eds = np.concatenate([arr_e, arr_d], axis=2)  # [8, 128, 2*TT]
    in_maps = []
    for k in range(N_CORES):
        in_maps.append({"eds": np.ascontiguousarray(eds[k])})
    return in_maps, B, Tc


def _make_consts(head: np.ndarray, tail: np.ndarray):
    iox = np.concatenate([np.arange(N_TYPES, dtype=np.float16),
                          np.arange(P, dtype=np.float16)])
    iox = np.tile(iox, (P, 1))
    W = np.concatenate([head, tail], axis=0).astype(np.float32)
    hi = W.astype(BF16)
    lo = (W - hi.astype(np.float32)).astype(BF16)
    tbl = np.zeros((WC, NCH, 257), BF16)
    for c in range(NCH):
        tbl[:, c, 0:128] = hi[c * WC:(c + 1) * WC]
        tbl[:, c, 128] = BF16(1.0)
        tbl[:, c, 129:257] = lo[c * WC:(c + 1) * WC]
    return {"iox": iox,
            "tbl": np.ascontiguousarray(tbl.reshape(WC, NCH * 257))}


def _get_runner(nc):
    """Cached jitted SPMD executor."""
    key = id(nc)
    if key in _runner_cache:
        return _runner_cache[key]
    import jax
    from jax.experimental.shard_map import shard_map
    from jax.sharding import Mesh, PartitionSpec
    from concourse import bass2jax
    from concourse.bass2jax import _bass_exec_p, partition_id_tensor

    bass2jax.install_neuronx_cc_hook()

    in_names, out_names, out_avals, zero_shapes = [], [], [], []
    for alloc in nc.m.functions[0].allocations:
        if not isinstance(alloc, mybir.MemoryLocationSet):
            continue
        name = alloc.memorylocations[0].name
        if alloc.kind == "ExternalInput":
            if nc.partition_id_tensor is None or name != nc.partition_id_tensor.name:
                in_names.append(name)
        elif alloc.kind == "ExternalOutput":
            shape = tuple(alloc.tensor_shape)
            dtype = mybir.dt.np(alloc.dtype)
            out_names.append(name)
            out_avals.append(jax.core.ShapedArray(shape, dtype))
            zero_shapes.append((shape, dtype))
    n_params = len(in_names)
    all_names = list(in_names) + list(out_names)
    if nc.partition_id_tensor is not None:
        all_names.append(nc.partition_id_tensor.name)
    donate = tuple(range(n_params, n_params + len(out_names)))

    def _body(*args):
        operands = list(args)
        if nc.partition_id_tensor is not None:
            operands.append(partition_id_tensor())
        outs = _bass_exec_p.bind(
            *operands,
            out_avals=tuple(out_avals),
            in_names=tuple(all_names),
            out_names=tuple(out_names),
            lowering_input_output_aliases=(),
            sim_require_finite=True,
            sim_require_nnan=True,
            nc=nc,
        )
        return tuple(outs)

    devices = jax.devices()[:N_CORES]
    mesh = Mesh(np.asarray(devices), ("core",))
    in_specs = (PartitionSpec("core"),) * (n_params + len(out_names))
    out_specs = (PartitionSpec("core"),) * len(out_names)
    fn = jax.jit(
        shard_map(_body, mesh=mesh, in_specs=in_specs, out_specs=out_specs,
                  check_rep=False),
        donate_argnums=donate, keep_unused=True,
    )
    r = (fn, in_names, out_names, out_avals, zero_shapes)
    _runner_cache[key] = r
    return r


class _Res:
    def __init__(self, results):
        self.results = results


def _run_spmd_cached(nc, in_maps):
    fn, in_names, out_names, out_avals, zero_shapes = _get_runner(nc)
    concat_in = [np.concatenate([m[n] for m in in_maps], axis=0)
                 for n in in_names]
    concat_zeros = [np.zeros((N_CORES * s[0], *s[1:]), d)
                    for s, d in zero_shapes]
    out_arrs = fn(*concat_in, *concat_zeros)
    results = []
    for c in range(N_CORES):
        results.append({
            name: np.asarray(out_arrs[i]).reshape(N_CORES, *out_avals[i].shape)[c]
            for i, name in enumerate(out_names)
        })
    return _Res(results)


def kernel(etypes, dst, rel_head_emb, rel_tail_emb, n_nodes):
    et = np.asarray(etypes).astype(np.int64)
    d = np.asarray(dst).astype(np.int64)
    head = np.asarray(rel_head_emb, dtype=np.float32)
    tail = np.asarray(rel_tail_emb, dtype=np.float32)
    nn = int(n_nodes)

    in_maps, B, Tc = _host_prepare(et, d, nn)
    consts = _make_consts(head, tail)
    for m in in_maps:
        m.update(consts)

    if (B, Tc, 1) not in _prog_cache:
        _prog_cache[(B, Tc, 1)] = _build_program(B, Tc)
    nc = _prog_cache[(B, Tc, 1)]

    import time as _time
    _t0 = _time.perf_counter()
    res = _run_spmd_cached(nc, in_maps)
    global LAST_DEVICE_WALL
    LAST_DEVICE_WALL = _time.perf_counter() - _t0

    out = np.concatenate([res.results[k]["feat"] for k in range(N_CORES)],
                         axis=0)
    return out[:nn]

